# revision 52
# baseline (speedup 1.0000x reference)
"""Multi-head attention (nonstandard softmax normalization) on 8 Trainium2 cores.

Reference computation (B=4, E=1024, S=1024, H=16, HS=64):
  per (b, h):  q = Wq[h] @ Q_h,  k = Wk[h] @ K_h,  v = Wv[h] @ V_h   (feature-first [HS, S])
               pre[s,t] = q[:,s]. k[:,t] / 8
               e = exp(pre);  denom[t] = sum_u e[t,u];  post[s,t] = e[s,t] / denom[t]
               out_h = v @ post.T                                     ([HS, S])
  out = concat_h(out_h);  result[b] = Wo @ out[b]

Sharding: core c -> (b = c//2, head-group hg = c%2 of 8 heads).  Each core
computes its 8 heads end-to-end plus the partial Wo product over its 512
rows; the host sums the two partial products per batch.

On-chip layout notes:
 - heads are processed in pairs; per-pair blockdiag(WT_h0, WT_h1) [128,128]
   weight tiles let the projections contract over the full 128 partitions.
 - QK^T is computed transposed (preT[t,s]) so exp(preT) ("E") has t on
   partitions, which is what the AV matmul needs; the two heads of a pair
   run as concurrent row-tiled (K=64) matmuls.
 - the denominator (partition-dim sums of E) falls out of the AV matmul
   for free: v^T carries an appended all-ones weight column, so PSUM row
   64 accumulates sum_t E[t,s] = denom[s] alongside the 64 feature rows.
   recip(denom) is then replicated across 64 partitions with a K=1 ones
   matmul into the unused rows 64:128 of the same PSUM bank, staged to
   SBUF, and folded into the PSUM->SBUF eviction of the AV result as a
   tensor_tensor multiply.
 - the final Wo projection is interleaved with the last pair's AV so the
   serial tail is one s-tile of Wo rather than the whole phase.
 - matmuls run as float32r (full-rate fp32) by bitcasting the operands.
"""

import os
import sys
import types

import numpy as np

import concourse.bass as bass
import concourse.mybir as mybir
import concourse.tile as tile
from contextlib import ExitStack

B, E, S_FULL, H = 4, 1024, 1024, 16
HS = 64
N_CORES = 8
HEADS_PER_CORE = H // 2          # 8: head-group per core
N_PAIRS_FULL = HEADS_PER_CORE // 2  # 4

_f32 = mybir.dt.float32


def _install_ntff_shim():
    """Register the axon NTFF profile hook if the image's antenv lacks it.

    Lets run_bass_kernel_spmd(trace=True) return exec_time_ns. Harmless if
    already present.
    """
    try:
        import antenv.axon_hooks  # noqa: F401
        return
    except ImportError:
        pass
    try:
        import antenv
        from trn_agent_boot.trn_boot import _ntff_profile_via_ctypes
    except ImportError:
        return
    mod = types.ModuleType("antenv.axon_hooks")
    mod._hook = None

    def set_axon_ntff_profile_hook(h):
        mod._hook = h

    def get_axon_ntff_profile_hook():
        return mod._hook

    mod.set_axon_ntff_profile_hook = set_axon_ntff_profile_hook
    mod.get_axon_ntff_profile_hook = get_axon_ntff_profile_hook
    sys.modules["antenv.axon_hooks"] = mod
    antenv.axon_hooks = mod
    for so in ("/opt/axon/libaxon_pjrt.so",):
        if os.path.exists(so):
            try:
                mod._hook = _ntff_profile_via_ctypes(so)
            except Exception:
                mod._hook = None
            break


def _install_drain_patch():
    """Work around this toolchain's walrus rejecting sem waits on Drain.

    TileContext's final drain carries end-of-kernel semaphore waits inline;
    this walrus build encodes Drain as NEURON_ISA_TPB_CTRL_NO_STRUCT and
    fails codegen ("Too many sync wait commands") for ANY inline wait.
    Equivalent semantics: emit the waits as standalone sync-engine wait
    instructions and leave the Drain bare.
    """
    if getattr(tile.TileContext, "_drain_patch_installed", False):
        return
    from concourse.vector_clock import ScopedClock

    def _patched_drain_and_barrier(self, tick_clock, wait_clock):
        drain_inst = self.nc.sync.drain()
        wait_clock.add_sem_waits(
            drain_inst.ins, ScopedClock({None: tick_clock.global_clock})
        )
        si = drain_inst.ins.sync_info
        waits = list(si.on_wait) if si is not None else []
        if waits:
            drain_inst.ins.sync_info = mybir.SyncInfo(
                on_wait=[], on_update=list(si.on_update) if si.on_update else []
            )
            by_name = (
                {h.name: h for h in self.sems.allocated().values()}
                if self.sems is not None else {}
            )
            for w in waits:
                sem = by_name.get(w.ant_name)
                assert sem is not None, f"unknown drain-wait sem: {w.ant_name}"
                assert w.wait_mode == "sem-ge-imm", w
                self.nc.sync.wait_ge(sem, w.wait_value)
        self.nc.all_engine_barrier()
        assert self.sems is not None
        popped = self.nc._tile_sem_poison_stack.pop()
        assert popped is self._sem_poison
        self.nc.clear_and_free_semaphores(list(self.sems.allocated().values()))
        self.nc.all_engine_barrier()

    tile.TileContext._drain_and_barrier = _patched_drain_and_barrier

    # Same walrus limitation, general form: at most ONE inline sem wait per
    # instruction. Tile's wait assignment can attach several (e.g. a DMA
    # waiting on a slot freed by PE + DVE + another queue). Hoist all but
    # the last wait onto same-engine EventSemaphore carrier instructions.
    orig_add = tile.TileContext._add_instruction

    def _split_add_instruction(self, inst):
        si = inst.sync_info
        if si is not None and si.on_wait and len(si.on_wait) > 1:
            waits = list(si.on_wait)
            for w in waits[:-1]:
                ev = mybir.InstEventSemaphore(
                    name=self.nc.get_next_instruction_name(),
                    engine=inst.engine,
                    sync_info=mybir.SyncInfo(on_wait=[w], on_update=[]),
                )
                orig_add(self, ev)
            inst.sync_info = mybir.SyncInfo(
                on_wait=[waits[-1]],
                on_update=list(si.on_update) if si.on_update else [],
            )
        orig_add(self, inst)

    tile.TileContext._add_instruction = _split_add_instruction
    tile.TileContext._drain_patch_installed = True


def build_core_kernel(S=1024, n_pairs=4, e_out=1024, mm_dt=mybir.dt.float16,
                      e_dt=mybir.dt.float16):
    """Build the per-core Bass program (SPMD: same program on all cores)."""
    _install_drain_patch()
    C = S // 128            # t-chunks (contiguous: t = c*128 + p)
    NT = min(512, S)        # matmul moving free-dim tile
    NS = S // NT            # s-tiles
    S4 = S // 4             # denom col-group width (<=512)
    EC = e_out // 128       # output e-chunks
    FP = n_pairs * 128      # feature rows handled by this core
    f32 = _f32

    rnd = mm_dt == mybir.dt.float32r
    # fp32r inputs are pre-rounded on the host (RNE, 12 dropped mantissa
    # bits) so they can be declared float32r directly; fp16 inputs are
    # converted on the host.
    in_dt = mm_dt

    nc = bass.Bass()
    q_rows = nc.declare_dram_parameter("q_rows", [FP, S], in_dt, isOutput=False)
    k_rows = nc.declare_dram_parameter("k_rows", [FP, S], in_dt, isOutput=False)
    v_rows = nc.declare_dram_parameter("v_rows", [FP, S], in_dt, isOutput=False)
    wqT = nc.declare_dram_parameter("wqT", [n_pairs, 128, 128], in_dt, isOutput=False)
    wkT = nc.declare_dram_parameter("wkT", [n_pairs, 128, 128], in_dt, isOutput=False)
    wvT = nc.declare_dram_parameter("wvT", [n_pairs, 128, 128], in_dt, isOutput=False)
    woT = nc.declare_dram_parameter("woT", [FP, e_out], in_dt, isOutput=False)
    out_part = nc.declare_dram_parameter("out_part", [e_out, S], f32, isOutput=True)

    Exp = mybir.ActivationFunctionType.Exp
    Mult = mybir.AluOpType.mult

    def mmv(ap):
        """View an operand AP with the matmul dtype."""
        return ap.bitcast(mm_dt) if rnd else ap

    with tile.TileContext(nc) as tc, ExitStack() as ctx:
        raws = ctx.enter_context(tc.tile_pool(name="raws", bufs=3))
        wop = ctx.enter_context(tc.tile_pool(name="wop", bufs=1))
        consts = ctx.enter_context(tc.tile_pool(name="consts", bufs=1))
        qks = ctx.enter_context(tc.tile_pool(name="qks", bufs=2))
        vts = ctx.enter_context(tc.tile_pool(name="vts", bufs=n_pairs))
        Epool = ctx.enter_context(tc.tile_pool(name="Epool", bufs=4))
        outp = ctx.enter_context(tc.tile_pool(name="outp", bufs=1))
        rcp = ctx.enter_context(tc.tile_pool(name="rcp", bufs=2))
        dap = ctx.enter_context(tc.tile_pool(name="dap", bufs=2))
        dstp = ctx.enter_context(tc.tile_pool(name="dstp", bufs=2))
        wostp = ctx.enter_context(tc.tile_pool(name="wostp", bufs=2))
        dram = ctx.enter_context(tc.tile_pool(name="dscratch", bufs=4, space="DRAM"))
        pqk = ctx.enter_context(tc.tile_pool(name="pqk", bufs=2, space="PSUM"))
        psm = ctx.enter_context(tc.tile_pool(name="psm", bufs=4, space="PSUM"))

        ones = consts.tile([128, 1], e_dt, tag="ones")
        nc.vector.memset(ones, 1.0)
        # preload the ACT exp table off the critical path (it costs ~1.3us
        # on the first real exp otherwise)
        warm = consts.tile([1, 8], f32, tag="warm")
        nc.vector.memset(warm, 0.0)
        nc.scalar.activation(out=warm, in_=warm, func=Exp, scale=1.0)
        wq_sb = consts.tile([128, n_pairs, 128], in_dt, tag="wq")
        wk_sb = consts.tile([128, n_pairs, 128], in_dt, tag="wk")
        wv_sb = consts.tile([128, n_pairs, 128], in_dt, tag="wv")
        w_dmas = [
            lambda: nc.sync.dma_start(out=wq_sb,
                                      in_=wqT.rearrange("r p m -> p r m")),
            lambda: nc.sync.dma_start(out=wk_sb,
                                      in_=wkT.rearrange("r p m -> p r m")),
            lambda: nc.sync.dma_start(out=wv_sb,
                                      in_=wvT.rearrange("r p m -> p r m")),
        ]

        st_dt = f32 if rnd else mm_dt
        q_all = qks.tile([128, n_pairs, S], st_dt, tag="qall")
        k_all = qks.tile([128, n_pairs, S], st_dt, tag="qall")
        out_all = outp.tile([128, n_pairs, S], st_dt, tag="outall")

        vt_tiles = [None] * n_pairs

        # ---- phase 1+2 (pipelined): load raw rows, project q/k/vT ----
        # Only pair 0 projects upfront; later pairs' loads + projections
        # interleave into earlier pairs' QK loops as dependency-free PE
        # filler (ACT starts sooner, PE bubbles get absorbed).
        def emit_loads(pr):
            qr = raws.tile([128, S], in_dt, tag="raw", name=f"qr_{pr}")
            kr = raws.tile([128, S], in_dt, tag="raw", name=f"kr_{pr}")
            vr = raws.tile([128, S], in_dt, tag="raw", name=f"vr_{pr}")
            nc.sync.dma_start(out=qr, in_=q_rows[pr * 128:(pr + 1) * 128, :])
            nc.sync.dma_start(out=kr, in_=k_rows[pr * 128:(pr + 1) * 128, :])
            nc.sync.dma_start(out=vr, in_=v_rows[pr * 128:(pr + 1) * 128, :])
            return qr, kr, vr

        def emit_proj_qk(pr, src, wt, dst):
            for st in range(NS):
                ps = psm.tile([128, NT], f32, tag="ps")
                nc.tensor.matmul(
                    ps,
                    lhsT=wt[:, pr, :],
                    rhs=src[:, st * NT:(st + 1) * NT],
                    start=True, stop=True,
                )
                nc.vector.tensor_copy(
                    out=mmv(dst[:, pr, st * NT:(st + 1) * NT]), in_=ps)

        def emit_proj_vt(pr, vr, c0, c1):
            # v^T directly: out[t, i] = sum_j V[j, t] * WvT[j, i]
            if vt_tiles[pr] is None:
                vt_tiles[pr] = vts.tile([128, C, 128], e_dt, tag="vt",
                                        name=f"vt_{pr}")
            vt = vt_tiles[pr]
            vrc = vr.rearrange("p (c t) -> p c t", c=C)
            for c in range(c0, c1):
                ps = psm.tile([128, NT], f32, tag="ps")
                nc.tensor.matmul(
                    ps[:, :128],
                    lhsT=vrc[:, c, :],
                    rhs=wv_sb[:, pr, :],
                    start=True, stop=True,
                )
                nc.vector.tensor_copy(out=vt[:, c, :], in_=ps[:, :128])

        def make_proj_thunks(pr, rtiles):
            qr, kr, vr = rtiles
            thunks = []
            if pr + 1 < n_pairs:
                thunks.append(lambda: rload.update({pr + 1: emit_loads(pr + 1)}))
            thunks += [
                lambda: emit_proj_qk(pr, qr, wq_sb, q_all),
                lambda: emit_proj_qk(pr, kr, wk_sb, k_all),
                lambda: emit_proj_vt(pr, vr, 0, C // 2),
                lambda: emit_proj_vt(pr, vr, C // 2, C),
            ]
            return thunks

        # pair-0 q/k rows first so the first projection's data leads the
        # weight transfers on the DMA queues
        rload = {0: emit_loads(0)}
        for w in w_dmas:
            w()
        rload[1] = emit_loads(1)
        emit_proj_qk(0, rload[0][0], wq_sb, q_all)
        emit_proj_qk(0, rload[0][1], wk_sb, k_all)
        emit_proj_vt(0, rload[0][2], 0, C)

        # woT in its own pool, loaded behind the raw inputs; the DMA overlaps
        # phase 3 and the Wo phase never waits on an E-pool slot
        woT_sb = wop.tile([128, n_pairs, e_out], in_dt, tag="woT")
        nc.sync.dma_start(out=woT_sb, in_=woT.rearrange("(f p) e -> p f e", p=128))

        # ---- phase 3 (per pair): QK^T, exp, denom, v-scale, AV ----
        # Software-pipelined: pair p's denom/AV matmuls are emitted between
        # pair p+1's QK chunks, so the PE stays dense (and HAM-warm) while
        # the ACT engine works through the exps.
        def emit_qk_chunk(pr, Es, kc, c):
            pst = [pqk.tile([128, NS * NT], f32, tag="pqk",
                            name=f"pqk_{pr}_{c}_{i}") for i in (0, 1)]
            # hh-major: head 0's exp input completes one matmul earlier,
            # starting the ACT pipeline sooner each chunk
            for hh in (0, 1):
                for st in range(NS):
                    nc.tensor.matmul(
                        pst[hh][:, st * NT:(st + 1) * NT],
                        lhsT=mmv(kc[hh][:, c, :]),
                        rhs=mmv(q_all[64 * hh:64 * hh + 64, pr,
                                      st * NT:(st + 1) * NT]),
                        start=True, stop=True,
                    )
                # E[t, s] = exp(preT[t, s] / 8)
                nc.scalar.activation(
                    out=Es[hh][:, c, :], in_=pst[hh][:],
                    func=Exp, scale=0.125)

        def emit_dacc_chunk(pr, Es, dtiles, dps_tiles, c):
            """DVE chunk-accumulation of the denominator: dacc += E[:, c, :]
            (f16 accumulator: 2x DVE rate, ~6e-4 relative error). After the
            final chunk, 4 quadrant ones-matmuls turn the partition sums
            into the dps rows the recip path reads -- 1k PE cols per
            (pair, head) instead of a full E pass."""
            Add = mybir.AluOpType.add
            for hh in (0, 1):
                with nc.allow_low_precision(reason="f16 denom accumulation"):
                    if c == 1:
                        dtiles[hh] = dap.tile([128, S], e_dt, tag="dacc",
                                              name=f"dacc_{pr}_{hh}")
                        nc.vector.tensor_tensor(
                            out=dtiles[hh], in0=Es[hh][:, 0, :],
                            in1=Es[hh][:, 1, :], op=Add)
                    else:
                        nc.vector.tensor_tensor(
                            out=dtiles[hh], in0=dtiles[hh],
                            in1=Es[hh][:, c, :], op=Add)
                if c == C - 1:
                    dps_tiles[hh] = psm.tile(
                        [128, NT], f32, tag="ps", name=f"dps_{pr}_{hh}")
                    for q4 in range(4):
                        nc.tensor.matmul(
                            dps_tiles[hh][32 * q4:32 * q4 + 1, :S4],
                            lhsT=ones,
                            rhs=dtiles[hh][:, q4 * S4:(q4 + 1) * S4],
                            start=True, stop=True,
                            tile_position=(0, 32 * q4),
                            skip_group_check=True,
                        )

        def make_av_thunks(pr, Es, dps_tiles):
            """Emission thunks for pair pr's recip + v-scale + AV."""
            thunks = []

            last = pr == n_pairs - 1

            def recip_scale(pr=pr):
                for hh in (0, 1):
                    dps = dps_tiles[hh]
                    dstage = dstp.tile([128, S4], f32, tag="dstage",
                                       name=f"dstage_{pr}_{hh}")
                    for q4 in range(4):
                        # the last pair's recip chain is latency-critical:
                        # run its evictions on the then-idle ACT engine
                        (nc.scalar.copy if last else nc.vector.tensor_copy)(
                            out=dstage[32 * q4:32 * q4 + 1, :],
                            in_=dps[32 * q4:32 * q4 + 1, :S4])
                    scr = dram.tile([S], f32, tag="scr", name=f"scr_{pr}_{hh}")
                    nc.sync.dma_start(
                        out=scr.rearrange("(a f) -> a f", f=S4),
                        in_=dstage.rearrange("(a b) f -> a b f", b=32)[:, 0, :])
                    rcr = rcp.tile([128, C], f32, tag="rcraw",
                                   name=f"rcr_{pr}_{hh}")
                    rc = rcp.tile([128, C], f32, tag="rc", name=f"rc_{pr}_{hh}")
                    nc.sync.dma_start(
                        out=rcr, in_=scr.rearrange("(c p) -> p c", p=128))
                    nc.vector.reciprocal(out=rc, in_=rcr)
                    # fold recip(denom) into v^T (per-partition broadcast);
                    # gpsimd is idle, so this never queues behind the DVE
                    # backlog on the latency-critical recip chain
                    vt = vt_tiles[pr]
                    nc.gpsimd.tensor_tensor(
                        out=vt[:, :, 64 * hh:64 * hh + 64],
                        in0=vt[:, :, 64 * hh:64 * hh + 64],
                        in1=rc[:, :, None].to_broadcast((128, C, 64)),
                        op=Mult,
                    )

            thunks.append(recip_scale)

            def av_mm(st, c0, c1, pr=pr, Es=Es):
                if st not in avps:
                    avps[st] = psm.tile([128, NT], f32, tag="ps",
                                        name=f"avp_{pr}_{st}")
                avp = avps[st]
                for c in range(c0, c1):
                    for hh in (0, 1):
                        nc.tensor.matmul(
                            avp[64 * hh:64 * hh + 64, :],
                            lhsT=vt_tiles[pr][:, c, 64 * hh:64 * hh + 64],
                            rhs=Es[hh][:, c, st * NT:(st + 1) * NT],
                            start=(c == 0), stop=(c == C - 1),
                            tile_position=(0, 64 * hh),
                            skip_group_check=True,
                        )

            def av_copy(st, pr=pr):
                (nc.scalar.copy if last else nc.vector.tensor_copy)(
                    out=mmv(out_all[:, pr, st * NT:(st + 1) * NT]),
                    in_=avps[st])

            avps = {}
            CH = C // 2
            for st in range(NS):
                thunks.append(lambda st=st: av_mm(st, 0, CH))
                thunks.append(lambda st=st: av_mm(st, CH, C))
                thunks.append(lambda st=st: av_copy(st))
            return thunks

        wo_ops = {}

        def wo_acc(st, ec, fc0, fc1):
            """Accumulate Wo psum for (st, ec) over pair range [fc0, fc1)."""
            if (st, ec) not in wo_ops:
                wo_ops[(st, ec)] = psm.tile(
                    [128, NT], f32, tag="ps", name=f"wop_{st}_{ec}")
            ops = wo_ops[(st, ec)]
            for fc in range(fc0, fc1):
                nc.tensor.matmul(
                    ops,
                    lhsT=woT_sb[:, fc, ec * 128:(ec + 1) * 128],
                    rhs=mmv(out_all[:, fc, st * NT:(st + 1) * NT]),
                    start=(fc == 0), stop=(fc == n_pairs - 1),
                )

        def wo_st(st, early=0):
            """Partial Wo projection for one s-tile; EC chunks staged into
            one SBUF buffer, written with two batched DMAs (the first half
            flushes while the second half is still being computed)."""
            wost = wostp.tile([128, EC, NT], f32, tag="wost")
            dst = out_part.rearrange("(ec p) s -> p ec s", p=128)
            sl = slice(st * NT, (st + 1) * NT)
            for ec in range(EC):
                wo_acc(st, ec, n_pairs - 1 if ec < early else 0, n_pairs)
                # ACT is idle during the drain (exps done) -- evict there so
                # the DVE queue never delays the last AV eviction
                nc.scalar.copy(out=wost[:, ec, :],
                               in_=wo_ops.pop((st, ec)))
                if ec == EC // 2 - 1:
                    nc.sync.dma_start(out=dst[:, :EC // 2, sl],
                                      in_=wost[:, :EC // 2, :])
            nc.sync.dma_start(out=dst[:, EC // 2:, sl],
                              in_=wost[:, EC // 2:, :])

        pending = make_proj_thunks(1, rload[1])
        for pr in range(n_pairs):
            E0 = Epool.tile([128, C, S], e_dt, tag="E", name=f"E0_{pr}")
            E1 = Epool.tile([128, C, S], e_dt, tag="E", name=f"E1_{pr}")
            Es = (E0, E1)
            kc = [
                k_all[64 * hh:64 * hh + 64, pr, :].rearrange(
                    "p (c t) -> p c t", c=C)
                for hh in (0, 1)
            ]
            dtiles = {}
            dps_tiles = {}
            n = len(pending)
            for c in range(C):
                emit_qk_chunk(pr, Es, kc, c)
                # lag the denom accumulation one chunk behind the exp
                if c >= 2:
                    emit_dacc_chunk(pr, Es, dtiles, dps_tiles, c - 1)
                for th in pending[(n * c) // C:(n * (c + 1)) // C]:
                    th()
            emit_dacc_chunk(pr, Es, dtiles, dps_tiles, C - 1)
            pending = make_av_thunks(pr, Es, dps_tiles)
            if pr + 2 < n_pairs:
                pending = pending + make_proj_thunks(pr + 2, rload[pr + 2])

        # drain the last pair, interleaving the Wo s-tiles so only the final
        # s-tile's Wo is a serial tail. The first few Wo chunks accumulate
        # pairs 0..2 during the recip chain (DVE/DMA) to keep the PE fed:
        # [recip, woe x3, av0a, av0b, copy0, wo(0), av1a, av1b, copy1, wo(1)]
        W_EARLY = 3
        pending[0]()
        for ec in range(W_EARLY):
            wo_acc(0, ec, 0, n_pairs - 1)
        for st in range(NS):
            for th in pending[1 + 3 * st:4 + 3 * st]:
                th()
            wo_st(st, early=W_EARLY if st == 0 else 0)

    return nc


def make_in_maps(queries, keys, values, Wq, Wk, Wv, Wo, mode="fp16"):
    """Shard the full inputs into the 8 per-core input dicts."""
    queries = np.ascontiguousarray(queries, dtype=np.float32)
    keys = np.ascontiguousarray(keys, dtype=np.float32)
    values = np.ascontiguousarray(values, dtype=np.float32)
    Wq = np.asarray(Wq, dtype=np.float32)
    Wk = np.asarray(Wk, dtype=np.float32)
    Wv = np.asarray(Wv, dtype=np.float32)
    Wo = np.asarray(Wo, dtype=np.float32)
    WoT = np.ascontiguousarray(Wo.T)

    def blockdiag(W, head_base):
        blk = np.zeros((N_PAIRS_FULL, 128, 128), dtype=np.float32)
        for pr in range(N_PAIRS_FULL):
            h0 = head_base + 2 * pr
            blk[pr, :64, :64] = W[h0].T
            blk[pr, 64:, 64:] = W[h0 + 1].T
        return blk

    def blockdiag_split(W, head_base):
        """Per-head masked halves: side s holds only head (2*pr + s)'s block."""
        blk = np.zeros((N_PAIRS_FULL, 2, 128, 128), dtype=np.float32)
        for pr in range(N_PAIRS_FULL):
            h0 = head_base + 2 * pr
            blk[pr, 0, :64, :64] = W[h0].T
            blk[pr, 1, 64:, 64:] = W[h0 + 1].T
        return blk

    in_maps = []
    for c in range(N_CORES):
        b, hg = c // 2, c % 2
        r0, r1 = hg * 512, (hg + 1) * 512
        head_base = hg * HEADS_PER_CORE
        m = {
            "q_rows": np.ascontiguousarray(queries[b, r0:r1, :]),
            "k_rows": np.ascontiguousarray(keys[b, r0:r1, :]),
            "v_rows": np.ascontiguousarray(values[b, r0:r1, :]),
            "wqT": blockdiag(Wq, head_base),
            "wkT": blockdiag(Wk, head_base),
            "wvT": blockdiag(Wv, head_base),
            "woT": np.ascontiguousarray(WoT[r0:r1, :]),
        }
        if mode == "fp32r":
            m = {k: round_fp32r(v) for k, v in m.items()}
        elif mode == "fp16":
            m = {k: v.astype(np.float16) for k, v in m.items()}
        in_maps.append(m)
    return in_maps


def round_fp32r(a):
    """Round-to-nearest-even with the low 12 mantissa bits dropped -- the
    exact rounding TRN2 fp32r applies (verified bit-exact against the DVE's
    fp32r-output copy)."""
    bits = np.ascontiguousarray(a, dtype=np.float32).view(np.uint32).astype(np.uint64)
    drop = 12
    mask = np.uint64((0xFFFFFFFF >> drop) << drop)
    half = np.uint64(1 << (drop - 1))
    lsb = (bits >> np.uint64(drop)) & np.uint64(1)
    rem = bits & np.uint64((1 << drop) - 1)
    up = (rem > half) | ((rem == half) & (lsb == 1))
    out = ((bits & mask) + np.where(up, np.uint64(1 << drop), np.uint64(0)))
    return out.astype(np.uint32).view(np.float32).reshape(np.asarray(a).shape)


LAST_RESULT = None


def kernel(queries, keys, values, Wq, Wk, Wv, Wo):
    """Full-input entry point: shard -> run on 8 NeuronCores -> unshard."""
    global LAST_RESULT
    from concourse.bass_utils import run_bass_kernel_spmd

    trace = bool(int(os.environ.get("BASS_KERNEL_TRACE", "0")))
    if trace:
        _install_ntff_shim()

    mode = os.environ.get("BASS_MM_MODE", "fp16")
    if mode == "fp32r":
        mm_dt, e_dt = mybir.dt.float32r, mybir.dt.bfloat16
    else:
        mm_dt, e_dt = mybir.dt.float16, mybir.dt.float16
    nc = build_core_kernel(S=S_FULL, n_pairs=N_PAIRS_FULL, e_out=E,
                           mm_dt=mm_dt, e_dt=e_dt)
    in_maps = make_in_maps(queries, keys, values, Wq, Wk, Wv, Wo, mode=mode)
    res = run_bass_kernel_spmd(nc, in_maps, core_ids=list(range(N_CORES)),
                               trace=trace)
    LAST_RESULT = res
    parts = [res.results[c]["out_part"] for c in range(N_CORES)]
    out = np.empty((B, E, S_FULL), dtype=np.float32)
    for b in range(B):
        out[b] = parts[2 * b] + parts[2 * b + 1]
    return out



# revision 53
# speedup vs baseline: 1.0152x; 1.0152x over previous
"""Multi-head attention (nonstandard softmax normalization) on 8 Trainium2 cores.

Reference computation (B=4, E=1024, S=1024, H=16, HS=64):
  per (b, h):  q = Wq[h] @ Q_h,  k = Wk[h] @ K_h,  v = Wv[h] @ V_h   (feature-first [HS, S])
               pre[s,t] = q[:,s]. k[:,t] / 8
               e = exp(pre);  denom[t] = sum_u e[t,u];  post[s,t] = e[s,t] / denom[t]
               out_h = v @ post.T                                     ([HS, S])
  out = concat_h(out_h);  result[b] = Wo @ out[b]

Sharding: core c -> (b = c//2, head-group hg = c%2 of 8 heads).  Each core
computes its 8 heads end-to-end plus the partial Wo product over its 512
rows; the host sums the two partial products per batch.

On-chip layout notes:
 - heads are processed in pairs; per-pair blockdiag(WT_h0, WT_h1) [128,128]
   weight tiles let the projections contract over the full 128 partitions.
 - QK^T is computed transposed (preT[t,s]) so exp(preT) ("E") has t on
   partitions, which is what the AV matmul needs; the two heads of a pair
   run as concurrent row-tiled (K=64) matmuls.
 - the denominator (partition-dim sums of E) falls out of the AV matmul
   for free: v^T carries an appended all-ones weight column, so PSUM row
   64 accumulates sum_t E[t,s] = denom[s] alongside the 64 feature rows.
   recip(denom) is then replicated across 64 partitions with a K=1 ones
   matmul into the unused rows 64:128 of the same PSUM bank, staged to
   SBUF, and folded into the PSUM->SBUF eviction of the AV result as a
   tensor_tensor multiply.
 - the final Wo projection is interleaved with the last pair's AV so the
   serial tail is one s-tile of Wo rather than the whole phase.
 - matmuls run as float32r (full-rate fp32) by bitcasting the operands.
"""

import os
import sys
import types

import numpy as np

import concourse.bass as bass
import concourse.mybir as mybir
import concourse.tile as tile
from contextlib import ExitStack

B, E, S_FULL, H = 4, 1024, 1024, 16
HS = 64
N_CORES = 8
HEADS_PER_CORE = H // 2          # 8: head-group per core
N_PAIRS_FULL = HEADS_PER_CORE // 2  # 4

_f32 = mybir.dt.float32


def _install_ntff_shim():
    """Register the axon NTFF profile hook if the image's antenv lacks it.

    Lets run_bass_kernel_spmd(trace=True) return exec_time_ns. Harmless if
    already present.
    """
    try:
        import antenv.axon_hooks  # noqa: F401
        return
    except ImportError:
        pass
    try:
        import antenv
        from trn_agent_boot.trn_boot import _ntff_profile_via_ctypes
    except ImportError:
        return
    mod = types.ModuleType("antenv.axon_hooks")
    mod._hook = None

    def set_axon_ntff_profile_hook(h):
        mod._hook = h

    def get_axon_ntff_profile_hook():
        return mod._hook

    mod.set_axon_ntff_profile_hook = set_axon_ntff_profile_hook
    mod.get_axon_ntff_profile_hook = get_axon_ntff_profile_hook
    sys.modules["antenv.axon_hooks"] = mod
    antenv.axon_hooks = mod
    for so in ("/opt/axon/libaxon_pjrt.so",):
        if os.path.exists(so):
            try:
                mod._hook = _ntff_profile_via_ctypes(so)
            except Exception:
                mod._hook = None
            break


def _install_drain_patch():
    """Work around this toolchain's walrus rejecting sem waits on Drain.

    TileContext's final drain carries end-of-kernel semaphore waits inline;
    this walrus build encodes Drain as NEURON_ISA_TPB_CTRL_NO_STRUCT and
    fails codegen ("Too many sync wait commands") for ANY inline wait.
    Equivalent semantics: emit the waits as standalone sync-engine wait
    instructions and leave the Drain bare.
    """
    if getattr(tile.TileContext, "_drain_patch_installed", False):
        return
    from concourse.vector_clock import ScopedClock

    def _patched_drain_and_barrier(self, tick_clock, wait_clock):
        drain_inst = self.nc.sync.drain()
        wait_clock.add_sem_waits(
            drain_inst.ins, ScopedClock({None: tick_clock.global_clock})
        )
        si = drain_inst.ins.sync_info
        waits = list(si.on_wait) if si is not None else []
        if waits:
            drain_inst.ins.sync_info = mybir.SyncInfo(
                on_wait=[], on_update=list(si.on_update) if si.on_update else []
            )
            by_name = (
                {h.name: h for h in self.sems.allocated().values()}
                if self.sems is not None else {}
            )
            for w in waits:
                sem = by_name.get(w.ant_name)
                assert sem is not None, f"unknown drain-wait sem: {w.ant_name}"
                assert w.wait_mode == "sem-ge-imm", w
                self.nc.sync.wait_ge(sem, w.wait_value)
        self.nc.all_engine_barrier()
        assert self.sems is not None
        popped = self.nc._tile_sem_poison_stack.pop()
        assert popped is self._sem_poison
        self.nc.clear_and_free_semaphores(list(self.sems.allocated().values()))
        self.nc.all_engine_barrier()

    tile.TileContext._drain_and_barrier = _patched_drain_and_barrier

    # Same walrus limitation, general form: at most ONE inline sem wait per
    # instruction. Tile's wait assignment can attach several (e.g. a DMA
    # waiting on a slot freed by PE + DVE + another queue). Hoist all but
    # the last wait onto same-engine EventSemaphore carrier instructions.
    orig_add = tile.TileContext._add_instruction

    def _split_add_instruction(self, inst):
        si = inst.sync_info
        if si is not None and si.on_wait and len(si.on_wait) > 1:
            waits = list(si.on_wait)
            for w in waits[:-1]:
                ev = mybir.InstEventSemaphore(
                    name=self.nc.get_next_instruction_name(),
                    engine=inst.engine,
                    sync_info=mybir.SyncInfo(on_wait=[w], on_update=[]),
                )
                orig_add(self, ev)
            inst.sync_info = mybir.SyncInfo(
                on_wait=[waits[-1]],
                on_update=list(si.on_update) if si.on_update else [],
            )
        orig_add(self, inst)

    tile.TileContext._add_instruction = _split_add_instruction
    tile.TileContext._drain_patch_installed = True


def build_core_kernel(S=1024, n_pairs=4, e_out=1024, mm_dt=mybir.dt.float16,
                      e_dt=mybir.dt.float16):
    """Build the per-core Bass program (SPMD: same program on all cores)."""
    _install_drain_patch()
    C = S // 128            # t-chunks (contiguous: t = c*128 + p)
    NT = min(512, S)        # matmul moving free-dim tile
    NS = S // NT            # s-tiles
    S4 = S // 4             # denom col-group width (<=512)
    EC = e_out // 128       # output e-chunks
    FP = n_pairs * 128      # feature rows handled by this core
    f32 = _f32

    rnd = mm_dt == mybir.dt.float32r
    # fp32r inputs are pre-rounded on the host (RNE, 12 dropped mantissa
    # bits) so they can be declared float32r directly; fp16 inputs are
    # converted on the host.
    in_dt = mm_dt

    nc = bass.Bass()
    q_rows = nc.declare_dram_parameter("q_rows", [FP, S], in_dt, isOutput=False)
    k_rows = nc.declare_dram_parameter("k_rows", [FP, S], in_dt, isOutput=False)
    v_rows = nc.declare_dram_parameter("v_rows", [FP, S], in_dt, isOutput=False)
    wqT = nc.declare_dram_parameter("wqT", [n_pairs, 128, 128], in_dt, isOutput=False)
    wkT = nc.declare_dram_parameter("wkT", [n_pairs, 128, 128], in_dt, isOutput=False)
    wvT = nc.declare_dram_parameter("wvT", [n_pairs, 128, 128], in_dt, isOutput=False)
    woT = nc.declare_dram_parameter("woT", [FP, e_out], in_dt, isOutput=False)
    out_part = nc.declare_dram_parameter("out_part", [e_out, S], f32, isOutput=True)

    Exp = mybir.ActivationFunctionType.Exp
    Mult = mybir.AluOpType.mult

    def mmv(ap):
        """View an operand AP with the matmul dtype."""
        return ap.bitcast(mm_dt) if rnd else ap

    with tile.TileContext(nc) as tc, ExitStack() as ctx:
        raws = ctx.enter_context(tc.tile_pool(name="raws", bufs=3))
        wop = ctx.enter_context(tc.tile_pool(name="wop", bufs=1))
        consts = ctx.enter_context(tc.tile_pool(name="consts", bufs=1))
        qks = ctx.enter_context(tc.tile_pool(name="qks", bufs=2))
        vts = ctx.enter_context(tc.tile_pool(name="vts", bufs=n_pairs))
        Epool = ctx.enter_context(tc.tile_pool(name="Epool", bufs=4))
        outp = ctx.enter_context(tc.tile_pool(name="outp", bufs=1))
        rcp = ctx.enter_context(tc.tile_pool(name="rcp", bufs=2))
        dap = ctx.enter_context(tc.tile_pool(name="dap", bufs=2))
        dstp = ctx.enter_context(tc.tile_pool(name="dstp", bufs=2))
        wostp = ctx.enter_context(tc.tile_pool(name="wostp", bufs=2))
        dram = ctx.enter_context(tc.tile_pool(name="dscratch", bufs=4, space="DRAM"))
        pqk = ctx.enter_context(tc.tile_pool(name="pqk", bufs=2, space="PSUM"))
        psm = ctx.enter_context(tc.tile_pool(name="psm", bufs=4, space="PSUM"))

        ones = consts.tile([128, 1], e_dt, tag="ones")
        nc.vector.memset(ones, 1.0)
        # preload the ACT exp table off the critical path (it costs ~1.3us
        # on the first real exp otherwise)
        warm = consts.tile([1, 8], f32, tag="warm")
        nc.vector.memset(warm, 0.0)
        nc.scalar.activation(out=warm, in_=warm, func=Exp, scale=1.0)
        wq_sb = consts.tile([128, n_pairs, 128], in_dt, tag="wq")
        wk_sb = consts.tile([128, n_pairs, 128], in_dt, tag="wk")
        wv_sb = consts.tile([128, n_pairs, 128], in_dt, tag="wv")
        nc.sync.dma_start(out=wq_sb, in_=wqT.rearrange("r p m -> p r m"))
        nc.sync.dma_start(out=wk_sb, in_=wkT.rearrange("r p m -> p r m"))
        nc.sync.dma_start(out=wv_sb, in_=wvT.rearrange("r p m -> p r m"))

        st_dt = f32 if rnd else mm_dt
        q_all = qks.tile([128, n_pairs, S], st_dt, tag="qall")
        k_all = qks.tile([128, n_pairs, S], st_dt, tag="qall")
        out_all = outp.tile([128, n_pairs, S], st_dt, tag="outall")

        vt_tiles = [None] * n_pairs

        # ---- phase 1+2 (pipelined): load raw rows, project q/k/vT ----
        # Only pair 0 projects upfront; later pairs' loads + projections
        # interleave into earlier pairs' QK loops as dependency-free PE
        # filler (ACT starts sooner, PE bubbles get absorbed).
        def emit_loads(pr):
            qr = raws.tile([128, S], in_dt, tag="raw", name=f"qr_{pr}")
            kr = raws.tile([128, S], in_dt, tag="raw", name=f"kr_{pr}")
            vr = raws.tile([128, S], in_dt, tag="raw", name=f"vr_{pr}")
            nc.sync.dma_start(out=qr, in_=q_rows[pr * 128:(pr + 1) * 128, :])
            nc.sync.dma_start(out=kr, in_=k_rows[pr * 128:(pr + 1) * 128, :])
            nc.sync.dma_start(out=vr, in_=v_rows[pr * 128:(pr + 1) * 128, :])
            return qr, kr, vr

        def emit_proj_qk(pr, src, wt, dst):
            for st in range(NS):
                ps = psm.tile([128, NT], f32, tag="ps")
                nc.tensor.matmul(
                    ps,
                    lhsT=wt[:, pr, :],
                    rhs=src[:, st * NT:(st + 1) * NT],
                    start=True, stop=True,
                )
                nc.vector.tensor_copy(
                    out=mmv(dst[:, pr, st * NT:(st + 1) * NT]), in_=ps)

        def emit_proj_vt(pr, vr, c0, c1):
            # v^T directly: out[t, i] = sum_j V[j, t] * WvT[j, i]
            if vt_tiles[pr] is None:
                vt_tiles[pr] = vts.tile([128, C, 128], e_dt, tag="vt",
                                        name=f"vt_{pr}")
            vt = vt_tiles[pr]
            vrc = vr.rearrange("p (c t) -> p c t", c=C)
            for c in range(c0, c1):
                ps = psm.tile([128, NT], f32, tag="ps")
                nc.tensor.matmul(
                    ps[:, :128],
                    lhsT=vrc[:, c, :],
                    rhs=wv_sb[:, pr, :],
                    start=True, stop=True,
                )
                nc.vector.tensor_copy(out=vt[:, c, :], in_=ps[:, :128])

        def make_proj_thunks(pr, rtiles):
            qr, kr, vr = rtiles
            thunks = []
            if pr + 1 < n_pairs:
                thunks.append(lambda: rload.update({pr + 1: emit_loads(pr + 1)}))
            thunks += [
                lambda: emit_proj_qk(pr, qr, wq_sb, q_all),
                lambda: emit_proj_qk(pr, kr, wk_sb, k_all),
                lambda: emit_proj_vt(pr, vr, 0, C // 2),
                lambda: emit_proj_vt(pr, vr, C // 2, C),
            ]
            return thunks

        rload = {0: emit_loads(0), 1: emit_loads(1)}
        emit_proj_qk(0, rload[0][0], wq_sb, q_all)
        emit_proj_qk(0, rload[0][1], wk_sb, k_all)
        emit_proj_vt(0, rload[0][2], 0, C)

        # woT in its own pool, loaded behind the raw inputs; the DMA overlaps
        # phase 3 and the Wo phase never waits on an E-pool slot
        woT_sb = wop.tile([128, n_pairs, e_out], in_dt, tag="woT")
        nc.sync.dma_start(out=woT_sb, in_=woT.rearrange("(f p) e -> p f e", p=128))

        # ---- phase 3 (per pair): QK^T, exp, denom, v-scale, AV ----
        # Software-pipelined: pair p's denom/AV matmuls are emitted between
        # pair p+1's QK chunks, so the PE stays dense (and HAM-warm) while
        # the ACT engine works through the exps.
        def emit_qk_chunk(pr, Es, kc, c):
            pst = [pqk.tile([128, NS * NT], f32, tag="pqk",
                            name=f"pqk_{pr}_{c}_{i}") for i in (0, 1)]
            for st in range(NS):
                for hh in (0, 1):
                    nc.tensor.matmul(
                        pst[hh][:, st * NT:(st + 1) * NT],
                        lhsT=mmv(kc[hh][:, c, :]),
                        rhs=mmv(q_all[64 * hh:64 * hh + 64, pr,
                                      st * NT:(st + 1) * NT]),
                        start=True, stop=True,
                    )
            for hh in (0, 1):
                # E[t, s] = exp(preT[t, s] / 8)
                nc.scalar.activation(
                    out=Es[hh][:, c, :], in_=pst[hh][:],
                    func=Exp, scale=0.125)

        def emit_dacc_chunk(pr, Es, dtiles, dps_tiles, c):
            """DVE chunk-accumulation of the denominator: dacc += E[:, c, :]
            (f16 accumulator: 2x DVE rate, ~6e-4 relative error). After the
            final chunk, 4 quadrant ones-matmuls turn the partition sums
            into the dps rows the recip path reads -- 1k PE cols per
            (pair, head) instead of a full E pass."""
            Add = mybir.AluOpType.add
            for hh in (0, 1):
                with nc.allow_low_precision(reason="f16 denom accumulation"):
                    if c == 1:
                        dtiles[hh] = dap.tile([128, S], e_dt, tag="dacc",
                                              name=f"dacc_{pr}_{hh}")
                        nc.vector.tensor_tensor(
                            out=dtiles[hh], in0=Es[hh][:, 0, :],
                            in1=Es[hh][:, 1, :], op=Add)
                    else:
                        nc.vector.tensor_tensor(
                            out=dtiles[hh], in0=dtiles[hh],
                            in1=Es[hh][:, c, :], op=Add)
                if c == C - 1:
                    dps_tiles[hh] = psm.tile(
                        [128, NT], f32, tag="ps", name=f"dps_{pr}_{hh}")
                    for q4 in range(4):
                        nc.tensor.matmul(
                            dps_tiles[hh][32 * q4:32 * q4 + 1, :S4],
                            lhsT=ones,
                            rhs=dtiles[hh][:, q4 * S4:(q4 + 1) * S4],
                            start=True, stop=True,
                            tile_position=(0, 32 * q4),
                            skip_group_check=True,
                        )

        def make_av_thunks(pr, Es, dps_tiles):
            """Emission thunks for pair pr's recip + v-scale + AV."""
            thunks = []

            last = pr == n_pairs - 1

            def recip_scale(pr=pr):
                for hh in (0, 1):
                    dps = dps_tiles[hh]
                    dstage = dstp.tile([128, S4], f32, tag="dstage",
                                       name=f"dstage_{pr}_{hh}")
                    for q4 in range(4):
                        # the last pair's recip chain is latency-critical:
                        # run its evictions on the then-idle ACT engine
                        (nc.scalar.copy if last else nc.vector.tensor_copy)(
                            out=dstage[32 * q4:32 * q4 + 1, :],
                            in_=dps[32 * q4:32 * q4 + 1, :S4])
                    scr = dram.tile([S], f32, tag="scr", name=f"scr_{pr}_{hh}")
                    nc.sync.dma_start(
                        out=scr.rearrange("(a f) -> a f", f=S4),
                        in_=dstage.rearrange("(a b) f -> a b f", b=32)[:, 0, :])
                    rcr = rcp.tile([128, C], f32, tag="rcraw",
                                   name=f"rcr_{pr}_{hh}")
                    rc = rcp.tile([128, C], f32, tag="rc", name=f"rc_{pr}_{hh}")
                    nc.sync.dma_start(
                        out=rcr, in_=scr.rearrange("(c p) -> p c", p=128))
                    nc.vector.reciprocal(out=rc, in_=rcr)
                    # fold recip(denom) into v^T (per-partition broadcast)
                    vt = vt_tiles[pr]
                    nc.vector.tensor_tensor(
                        out=vt[:, :, 64 * hh:64 * hh + 64],
                        in0=vt[:, :, 64 * hh:64 * hh + 64],
                        in1=rc[:, :, None].to_broadcast((128, C, 64)),
                        op=Mult,
                    )

            thunks.append(recip_scale)

            def av_mm(st, c0, c1, pr=pr, Es=Es):
                if st not in avps:
                    avps[st] = psm.tile([128, NT], f32, tag="ps",
                                        name=f"avp_{pr}_{st}")
                avp = avps[st]
                for c in range(c0, c1):
                    for hh in (0, 1):
                        nc.tensor.matmul(
                            avp[64 * hh:64 * hh + 64, :],
                            lhsT=vt_tiles[pr][:, c, 64 * hh:64 * hh + 64],
                            rhs=Es[hh][:, c, st * NT:(st + 1) * NT],
                            start=(c == 0), stop=(c == C - 1),
                            tile_position=(0, 64 * hh),
                            skip_group_check=True,
                        )

            def av_copy(st, pr=pr):
                (nc.scalar.copy if last else nc.vector.tensor_copy)(
                    out=mmv(out_all[:, pr, st * NT:(st + 1) * NT]),
                    in_=avps[st])

            avps = {}
            CH = C // 2
            for st in range(NS):
                thunks.append(lambda st=st: av_mm(st, 0, CH))
                thunks.append(lambda st=st: av_mm(st, CH, C))
                thunks.append(lambda st=st: av_copy(st))
            return thunks

        wo_ops = {}

        def wo_acc(st, ec, fc0, fc1):
            """Accumulate Wo psum for (st, ec) over pair range [fc0, fc1)."""
            if (st, ec) not in wo_ops:
                wo_ops[(st, ec)] = psm.tile(
                    [128, NT], f32, tag="ps", name=f"wop_{st}_{ec}")
            ops = wo_ops[(st, ec)]
            for fc in range(fc0, fc1):
                nc.tensor.matmul(
                    ops,
                    lhsT=woT_sb[:, fc, ec * 128:(ec + 1) * 128],
                    rhs=mmv(out_all[:, fc, st * NT:(st + 1) * NT]),
                    start=(fc == 0), stop=(fc == n_pairs - 1),
                )

        def wo_st(st, early=0):
            """Partial Wo projection for one s-tile; EC chunks staged into
            one SBUF buffer, written with two batched DMAs (the first half
            flushes while the second half is still being computed)."""
            wost = wostp.tile([128, EC, NT], f32, tag="wost")
            dst = out_part.rearrange("(ec p) s -> p ec s", p=128)
            sl = slice(st * NT, (st + 1) * NT)
            for ec in range(EC):
                wo_acc(st, ec, n_pairs - 1 if ec < early else 0, n_pairs)
                # ACT is idle during the drain (exps done) -- evict there so
                # the DVE queue never delays the last AV eviction
                nc.scalar.copy(out=wost[:, ec, :],
                               in_=wo_ops.pop((st, ec)))
                if ec == EC // 2 - 1:
                    nc.sync.dma_start(out=dst[:, :EC // 2, sl],
                                      in_=wost[:, :EC // 2, :])
            nc.sync.dma_start(out=dst[:, EC // 2:, sl],
                              in_=wost[:, EC // 2:, :])

        pending = make_proj_thunks(1, rload[1])
        for pr in range(n_pairs):
            E0 = Epool.tile([128, C, S], e_dt, tag="E", name=f"E0_{pr}")
            E1 = Epool.tile([128, C, S], e_dt, tag="E", name=f"E1_{pr}")
            Es = (E0, E1)
            kc = [
                k_all[64 * hh:64 * hh + 64, pr, :].rearrange(
                    "p (c t) -> p c t", c=C)
                for hh in (0, 1)
            ]
            dtiles = {}
            dps_tiles = {}
            n = len(pending)
            for c in range(C):
                emit_qk_chunk(pr, Es, kc, c)
                # lag the denom accumulation one chunk behind the exp
                if c >= 2:
                    emit_dacc_chunk(pr, Es, dtiles, dps_tiles, c - 1)
                for th in pending[(n * c) // C:(n * (c + 1)) // C]:
                    th()
            emit_dacc_chunk(pr, Es, dtiles, dps_tiles, C - 1)
            pending = make_av_thunks(pr, Es, dps_tiles)
            if pr + 2 < n_pairs:
                pending = pending + make_proj_thunks(pr + 2, rload[pr + 2])

        # drain the last pair, interleaving the Wo s-tiles so only the final
        # s-tile's Wo is a serial tail. The first few Wo chunks accumulate
        # pairs 0..2 during the recip chain (DVE/DMA) to keep the PE fed:
        # [recip, woe x3, av0a, av0b, copy0, wo(0), av1a, av1b, copy1, wo(1)]
        W_EARLY = 3
        pending[0]()
        for ec in range(W_EARLY):
            wo_acc(0, ec, 0, n_pairs - 1)
        for st in range(NS):
            for th in pending[1 + 3 * st:4 + 3 * st]:
                th()
            wo_st(st, early=W_EARLY if st == 0 else 0)

    return nc


def make_in_maps(queries, keys, values, Wq, Wk, Wv, Wo, mode="fp16"):
    """Shard the full inputs into the 8 per-core input dicts."""
    queries = np.ascontiguousarray(queries, dtype=np.float32)
    keys = np.ascontiguousarray(keys, dtype=np.float32)
    values = np.ascontiguousarray(values, dtype=np.float32)
    Wq = np.asarray(Wq, dtype=np.float32)
    Wk = np.asarray(Wk, dtype=np.float32)
    Wv = np.asarray(Wv, dtype=np.float32)
    Wo = np.asarray(Wo, dtype=np.float32)
    WoT = np.ascontiguousarray(Wo.T)

    def blockdiag(W, head_base):
        blk = np.zeros((N_PAIRS_FULL, 128, 128), dtype=np.float32)
        for pr in range(N_PAIRS_FULL):
            h0 = head_base + 2 * pr
            blk[pr, :64, :64] = W[h0].T
            blk[pr, 64:, 64:] = W[h0 + 1].T
        return blk

    def blockdiag_split(W, head_base):
        """Per-head masked halves: side s holds only head (2*pr + s)'s block."""
        blk = np.zeros((N_PAIRS_FULL, 2, 128, 128), dtype=np.float32)
        for pr in range(N_PAIRS_FULL):
            h0 = head_base + 2 * pr
            blk[pr, 0, :64, :64] = W[h0].T
            blk[pr, 1, 64:, 64:] = W[h0 + 1].T
        return blk

    in_maps = []
    for c in range(N_CORES):
        b, hg = c // 2, c % 2
        r0, r1 = hg * 512, (hg + 1) * 512
        head_base = hg * HEADS_PER_CORE
        m = {
            "q_rows": np.ascontiguousarray(queries[b, r0:r1, :]),
            "k_rows": np.ascontiguousarray(keys[b, r0:r1, :]),
            "v_rows": np.ascontiguousarray(values[b, r0:r1, :]),
            "wqT": blockdiag(Wq, head_base),
            "wkT": blockdiag(Wk, head_base),
            "wvT": blockdiag(Wv, head_base),
            "woT": np.ascontiguousarray(WoT[r0:r1, :]),
        }
        if mode == "fp32r":
            m = {k: round_fp32r(v) for k, v in m.items()}
        elif mode == "fp16":
            m = {k: v.astype(np.float16) for k, v in m.items()}
        in_maps.append(m)
    return in_maps


def round_fp32r(a):
    """Round-to-nearest-even with the low 12 mantissa bits dropped -- the
    exact rounding TRN2 fp32r applies (verified bit-exact against the DVE's
    fp32r-output copy)."""
    bits = np.ascontiguousarray(a, dtype=np.float32).view(np.uint32).astype(np.uint64)
    drop = 12
    mask = np.uint64((0xFFFFFFFF >> drop) << drop)
    half = np.uint64(1 << (drop - 1))
    lsb = (bits >> np.uint64(drop)) & np.uint64(1)
    rem = bits & np.uint64((1 << drop) - 1)
    up = (rem > half) | ((rem == half) & (lsb == 1))
    out = ((bits & mask) + np.where(up, np.uint64(1 << drop), np.uint64(0)))
    return out.astype(np.uint32).view(np.float32).reshape(np.asarray(a).shape)


LAST_RESULT = None


def kernel(queries, keys, values, Wq, Wk, Wv, Wo):
    """Full-input entry point: shard -> run on 8 NeuronCores -> unshard."""
    global LAST_RESULT
    from concourse.bass_utils import run_bass_kernel_spmd

    trace = bool(int(os.environ.get("BASS_KERNEL_TRACE", "0")))
    if trace:
        _install_ntff_shim()

    mode = os.environ.get("BASS_MM_MODE", "fp16")
    if mode == "fp32r":
        mm_dt, e_dt = mybir.dt.float32r, mybir.dt.bfloat16
    else:
        mm_dt, e_dt = mybir.dt.float16, mybir.dt.float16
    nc = build_core_kernel(S=S_FULL, n_pairs=N_PAIRS_FULL, e_out=E,
                           mm_dt=mm_dt, e_dt=e_dt)
    in_maps = make_in_maps(queries, keys, values, Wq, Wk, Wv, Wo, mode=mode)
    res = run_bass_kernel_spmd(nc, in_maps, core_ids=list(range(N_CORES)),
                               trace=trace)
    LAST_RESULT = res
    parts = [res.results[c]["out_part"] for c in range(N_CORES)]
    out = np.empty((B, E, S_FULL), dtype=np.float32)
    for b in range(B):
        out[b] = parts[2 * b] + parts[2 * b + 1]
    return out

